# revision 14
# baseline (speedup 1.0000x reference)
"""Multi-head attention (B=2, S=2048, D=1024, H=16) on 8 TRN2 NeuronCores.

Sharding: DP=2 over batch x TP=4 over heads (4 heads/core).
Per core: QKV projections for its 256 output dims, attention for its 4
heads on its batch, row-parallel output projection producing a partial
[2048, 1024]; host sums the 4 partials per batch and adds bo.

v2 layout strategy (per core):
  - host pre-transposes x (q/k/v) to xT [1024, 2048] bf16 and weights bf16
  - emit order tuned for overlap: k-proj, q-proj(first half), scores for
    the first (ic,hp) chunk (so ACT exp starts early), q-proj(rest),
    v-proj, then the attention steady loop with the output projection of
    each ic interleaved as soon as both head-pairs finish
  - qh/kh stored f32r (bf16 q/k caused max-err spikes), v/et/stacked/wo bf16
  - exp split: most j-tiles on ACT (exp, scale=1/8 folded); NDVE j-tiles
    per (hp,ic) computed on DVE via a two-term Schraudolph: two
    tensor_scalar ops fp32->int16 (RNE) whose int16 bit patterns are bf16
    exponent/mantissa encodings of ~2^(t)+2^(t-0.5), summed with one
    bf16 tensor_tensor add; rel err ~0.5% rms, consistent denominators
  - attnV via vh_aug [128, 65] (ones column -> denominator row 64)
  - normalization: reciprocal_approx_fast on denom rows, one K=2 matmul
    broadcasts both heads' 1/denom to 128 rows, one DVE copy + 2 muls
  - output projection accumulates in PSUM and DMAs PSUM->DRAM directly
"""
import numpy as np

B, S, D = 2, 2048, 1024
HEADS, DK = 16, 64
NCORES, DP, TP = 8, 2, 4
OPC = D // TP          # 256 output dims per core
HPC = HEADS // TP      # 4 heads per core
NDC = D // 128         # 8 contraction chunks
NST = S // 128         # 16 s-tiles (j tiles)
NIC = S // 512         # 4 i-chunks

# two-term Schraudolph constants (see sim_err.py calibration)
SCH_A = float(0.125 * 1.4426950408889634 * 128)
SCH_B1 = 16149.67
SCH_B2 = 16086.17
DVE_JS = (5, 10, 15)   # j-tiles whose exp runs on DVE

_cache = {}


def _build():
    import concourse.mybir as mybir
    import concourse.tile as tile
    from concourse import bacc

    F32 = mybir.dt.float32
    F32R = mybir.dt.float32r
    BF16 = mybir.dt.bfloat16
    I16 = mybir.dt.int16
    Exp = mybir.ActivationFunctionType.Exp
    Mult = mybir.AluOpType.mult
    Add = mybir.AluOpType.add

    nc = bacc.Bacc("TRN2", target_bir_lowering=False, debug=False)

    xq_d = nc.dram_tensor("xqt", [D, S], BF16, kind="ExternalInput")
    xk_d = nc.dram_tensor("xkt", [D, S], BF16, kind="ExternalInput")
    xv_d = nc.dram_tensor("xvt", [D, S], BF16, kind="ExternalInput")
    wq_d = nc.dram_tensor("wqt", [D, OPC], BF16, kind="ExternalInput")
    wk_d = nc.dram_tensor("wkt", [D, OPC], BF16, kind="ExternalInput")
    wv_d = nc.dram_tensor("wvt", [D, OPC], BF16, kind="ExternalInput")
    bq_d = nc.dram_tensor("bq", [2, 128, 1], F32, kind="ExternalInput")
    bk_d = nc.dram_tensor("bk", [2, 128, 1], F32, kind="ExternalInput")
    bv_d = nc.dram_tensor("bv", [128, OPC], F32, kind="ExternalInput")
    wo_d = nc.dram_tensor("wot", [2, 128, D], BF16, kind="ExternalInput")
    out_d = nc.dram_tensor("out", [S, D], BF16, kind="ExternalOutput")

    with tile.TileContext(nc) as tc:
        from contextlib import ExitStack
        es = ExitStack()
        with es:
            wp = es.enter_context(tc.tile_pool(name="wp", bufs=1))
            acts = es.enter_context(tc.tile_pool(name="acts", bufs=1))
            xp = es.enter_context(tc.tile_pool(name="xin", bufs=1))
            ep = es.enter_context(tc.tile_pool(name="ep", bufs=1))
            sps = es.enter_context(tc.tile_pool(name="sps", bufs=2, space="PSUM"))

            # constants
            ones164 = wp.tile([1, 64], BF16, name="ones164")
            nc.vector.memset(ones164[:], 1.0)

            # persistent activations
            qh_st = [acts.tile([128, S], F32R, name=f"qh{h}") for h in range(2)]
            kh_st = [acts.tile([128, S], F32R, name=f"kh{h}") for h in range(2)]
            # vh_aug: 16 j-tile blocks x 4 heads of [128, 65]; ones col at 64
            vh_all = acts.tile([128, NST * HPC * 65], BF16, name="vh_all")
            ones_cols = vh_all[:].rearrange("p (g c) -> p g c", c=65)[:, :, 64:65]
            nc.vector.memset(ones_cols, 1.0)
            stacked = [acts.tile([128, S], BF16, name=f"st{h}") for h in range(2)]

            def vh_ap(h, j):
                base = (j * HPC + h) * 65
                return vh_all[:, base:base + 65]

            # et tiles: generous buffering (prefill of (ic0,hp0) keeps 16 live)
            def et_tile():
                return ep.tile([128, 1024], BF16, name="et", tag="et", bufs=20)

            def s12_tile():
                return ep.tile([128, 1024], I16, name="s12", tag="s12", bufs=6)

            # ---------- DMA helpers ----------
            dma_rr = [0]

            def dma(dst, src):
                eng = nc.sync if dma_rr[0] % 2 == 0 else nc.gpsimd
                dma_rr[0] += 1
                eng.dma_start(dst, src)

            # ---------- phase helpers ----------
            def load_w(wd, n=NDC):
                wt = [wp.tile([128, OPC], BF16, name=f"w_{wd.name}_{i}")
                      for i in range(n)]
                for i in range(n):
                    dma(wt[i][:], wd.ap()[i * 128:(i + 1) * 128, :])
                return wt

            def load_x_chunks(xd, sc2):
                xt = [xp.tile([128, 1024], BF16, name="xt", tag="xt", bufs=18)
                      for _ in range(NDC)]
                for dc in range(NDC):
                    dma(xt[dc][:],
                        xd.ap()[dc * 128:(dc + 1) * 128,
                                sc2 * 1024:(sc2 + 1) * 1024])
                return xt

            def qk_proj(pool, xt, wt, bias, dest, sc2):
                # dest[hp][:, sc*512:(sc+1)*512] = wt.T @ xt + bias
                for half in range(2):
                    sc = sc2 * 2 + half
                    for hp in range(2):
                        p = pool.tile([128, 512], F32, name="pp", tag="pp")
                        for dc in range(NDC):
                            nc.tensor.matmul(
                                p[:], wt[dc][:, hp * 128:(hp + 1) * 128],
                                xt[dc][:, half * 512:(half + 1) * 512],
                                start=(dc == 0), stop=(dc == NDC - 1))
                        nc.vector.tensor_scalar_add(
                            dest[hp][:, sc * 512:(sc + 1) * 512], p[:],
                            bias[hp][:])

            def scores_pair(hp, ic, j):
                sp = sps.tile([128, 1024], F32, name="sp", tag="sp")
                nc.tensor.matmul(
                    sp[:, 0:512], kh_st[hp][0:64, j * 128:(j + 1) * 128],
                    qh_st[hp][0:64, ic * 512:(ic + 1) * 512],
                    start=True, stop=True, tile_position=(0, 0))
                nc.tensor.matmul(
                    sp[:, 512:1024], kh_st[hp][64:128, j * 128:(j + 1) * 128],
                    qh_st[hp][64:128, ic * 512:(ic + 1) * 512],
                    start=True, stop=True, tile_position=(64, 0))
                return sp

            def exp_tile(sp, j):
                et = et_tile()
                if j in DVE_JS:
                    s1 = s12_tile()
                    s2 = s12_tile()
                    nc.vector.tensor_scalar(s1[:], sp[:], SCH_A, SCH_B1,
                                            Mult, Add)
                    nc.vector.tensor_scalar(s2[:], sp[:], SCH_A, SCH_B2,
                                            Mult, Add)
                    nc.gpsimd.tensor_tensor(
                        et[:], s1[:].bitcast(BF16), s2[:].bitcast(BF16), Add)
                else:
                    nc.scalar.activation(et[:], sp[:], Exp, scale=0.125)
                return et

            def attn_v(av, hp, j, et):
                for h2 in range(2):
                    nc.tensor.matmul(
                        av[h2][0:DK + 1, :], vh_ap(hp * 2 + h2, j),
                        et[:, h2 * 512:(h2 + 1) * 512],
                        start=(j == 0), stop=(j == NST - 1),
                        skip_group_check=True)

            def norm(bp, av, hp, ic):
                # copy both heads' denominator rows (bf16), broadcast them to
                # 128 rows with two K=1 col-tiled matmuls, then reciprocal of
                # the broadcast (DVE time ~ free-dim only) and scale
                den = [ep.tile([1, 512], BF16, name="den", tag="den", bufs=4)
                       for _ in range(2)]
                for h2 in range(2):
                    nc.vector.tensor_copy(den[h2][:], av[h2][DK:DK + 1, :])
                r2 = bp.tile([128, 512], F32, name="r2", tag="mb")
                nc.tensor.matmul(r2[0:64, :], ones164[:], den[0][:],
                                 start=True, stop=True, tile_position=(0, 0))
                nc.tensor.matmul(r2[64:128, :], ones164[:], den[1][:],
                                 start=True, stop=True, tile_position=(0, 64))
                r2s = ep.tile([128, 512], F32, name="r2s", tag="r2s", bufs=2)
                nc.vector.reciprocal_approx_fast(r2s[:], r2[:])
                for h2 in range(2):
                    nc.vector.tensor_mul(
                        stacked[hp][h2 * 64:(h2 + 1) * 64,
                                    ic * 512:(ic + 1) * 512],
                        av[h2][0:DK, :], r2s[h2 * 64:(h2 + 1) * 64, :])

            def out_proj(bp, wo_t, ic):
                for it in range(ic * 4, ic * 4 + 4):
                    for mc in range(2):
                        po = bp.tile([128, 512], F32, name="po", tag="mb")
                        for hp in range(2):
                            nc.tensor.matmul(
                                po[:], stacked[hp][:, it * 128:(it + 1) * 128],
                                wo_t[hp][:, mc * 512:(mc + 1) * 512],
                                start=(hp == 0), stop=(hp == 1))
                        ot = ep.tile([128, 512], BF16, name="ot", tag="ot",
                                     bufs=4)
                        nc.vector.tensor_copy(ot[:], po[:])
                        dma(out_d.ap()[it * 128:(it + 1) * 128,
                                       mc * 512:(mc + 1) * 512], ot[:])

            # ================= phase A + prefill =================
            prefill_ets = []
            with tc.tile_pool(name="pps", bufs=3, space="PSUM") as pps:
                # k first (scores need all of kh), then q sc2=0
                wk_t = load_w(wk_d)
                bk_t = [wp.tile([128, 1], F32, name=f"bk{h}") for h in range(2)]
                for h in range(2):
                    dma(bk_t[h][:], bk_d.ap()[h])
                wq_t = load_w(wq_d)
                bq_t = [wp.tile([128, 1], F32, name=f"bq{h}") for h in range(2)]
                for h in range(2):
                    dma(bq_t[h][:], bq_d.ap()[h])
                for sc2 in range(2):
                    xt = load_x_chunks(xk_d, sc2)
                    qk_proj(pps, xt, wk_t, bk_t, kh_st, sc2)
                xt = load_x_chunks(xq_d, 0)
                qk_proj(pps, xt, wq_t, bq_t, qh_st, 0)

                # prefill: scores+exp for (ic=0, hp=0) while v-proj runs
                for j in range(NST):
                    sp = scores_pair(0, 0, j)
                    prefill_ets.append(exp_tile(sp, j))

                # q sc2=1
                xt = load_x_chunks(xq_d, 1)
                qk_proj(pps, xt, wq_t, bq_t, qh_st, 1)

                # v projection
                wv_t = load_w(wv_d)
                bv2 = wp.tile([128, OPC], F32, name="bv2")
                dma(bv2[:], bv_d.ap())
                for sc2 in range(2):
                    xt = load_x_chunks(xv_d, sc2)
                    for st8 in range(8):
                        st = sc2 * 8 + st8
                        pv = pps.tile([128, OPC], F32, name="pv", tag="pp")
                        for dc in range(NDC):
                            nc.tensor.matmul(
                                pv[:], xt[dc][:, st8 * 128:(st8 + 1) * 128],
                                wv_t[dc][:], start=(dc == 0),
                                stop=(dc == NDC - 1))
                        # scatter 4 heads' 64-dim slices into vh_all block st
                        dst = vh_all[:, st * HPC * 65:(st + 1) * HPC * 65]
                        dst = dst.rearrange("p (h c) -> p h c", h=HPC)[:, :, 0:64]
                        nc.vector.tensor_add(
                            dst, pv[:].rearrange("p (h c) -> p h c", h=HPC),
                            bv2[:].rearrange("p (h c) -> p h c", h=HPC))

            # wo (small, early in phase B)
            wo_t = [wp.tile([128, D], BF16, name=f"wo{h}") for h in range(2)]
            for h in range(2):
                dma(wo_t[h][:], wo_d.ap()[h])

            # ================= phase B/C steady loop =================
            # one 4-slot pool (1 bank/slot) shared by av accumulators, the
            # r2 broadcast, and the out-proj tiles: 4 banks + sps 4 = 8
            with tc.tile_pool(name="bp", bufs=4, space="PSUM") as bp:
                for ic in range(NIC):
                    for hp in range(2):
                        av = [bp.tile([DK + 1, 512], F32, name="av", tag="mb")
                              for _ in range(2)]
                        if ic == 0 and hp == 0:
                            for j in range(NST):
                                attn_v(av, hp, j, prefill_ets[j])
                                prefill_ets[j] = None
                        else:
                            ets = {}
                            for j in range(2):
                                ets[j] = exp_tile(scores_pair(hp, ic, j), j)
                            for j in range(NST):
                                attn_v(av, hp, j, ets.pop(j))
                                if j + 2 < NST:
                                    ets[j + 2] = exp_tile(
                                        scores_pair(hp, ic, j + 2), j + 2)
                        norm(bp, av, hp, ic)
                    out_proj(bp, wo_t, ic)

    nc.compile()
    return nc


def _prep_inputs(q, k, v, Wq, bq, Wk, bk, Wv, bv, Wo, bo):
    import ml_dtypes
    f = np.float32
    bf = ml_dtypes.bfloat16
    xT = {}
    for g in range(DP):
        xT[("q", g)] = np.ascontiguousarray(np.asarray(q[g], f).T.astype(bf))
        xT[("k", g)] = np.ascontiguousarray(np.asarray(k[g], f).T.astype(bf))
        xT[("v", g)] = np.ascontiguousarray(np.asarray(v[g], f).T.astype(bf))
    Wq, Wk, Wv, Wo = (np.asarray(a, f) for a in (Wq, Wk, Wv, Wo))
    bq, bk, bv = (np.asarray(a, f) for a in (bq, bk, bv))
    in_maps = []
    for c in range(NCORES):
        g, r = divmod(c, TP)
        sl = slice(r * OPC, (r + 1) * OPC)
        in_maps.append({
            "xqt": xT[("q", g)], "xkt": xT[("k", g)], "xvt": xT[("v", g)],
            "wqt": np.ascontiguousarray(Wq[sl].T.astype(bf)),
            "wkt": np.ascontiguousarray(Wk[sl].T.astype(bf)),
            "wvt": np.ascontiguousarray(Wv[sl].T.astype(bf)),
            "bq": bq[sl].reshape(2, 128, 1),
            "bk": bk[sl].reshape(2, 128, 1),
            "bv": np.ascontiguousarray(np.broadcast_to(bv[sl], (128, OPC))),
            "wot": np.ascontiguousarray(Wo[:, sl].T).reshape(2, 128, D).astype(bf),
        })
    return in_maps


def kernel(q, k, v, Wq, bq, Wk, bk, Wv, bv, Wo, bo, _trace=False):
    from concourse.bass_utils import run_bass_kernel_spmd

    if "nc" not in _cache:
        _cache["nc"] = _build()
    nc = _cache["nc"]
    in_maps = _prep_inputs(q, k, v, Wq, bq, Wk, bk, Wv, bv, Wo, bo)
    res = run_bass_kernel_spmd(nc, in_maps, list(range(NCORES)), trace=_trace)
    _cache["last_exec_time_ns"] = res.exec_time_ns
    _cache["last_res"] = res
    parts = [res.results[c]["out"] for c in range(NCORES)]
    bo = np.asarray(bo, np.float32)
    out = np.empty((B, S, D), np.float32)
    for g in range(DP):
        acc = parts[g * TP].astype(np.float32)
        for r in range(1, TP):
            acc = acc + parts[g * TP + r]
        out[g] = acc + bo
    return out


# revision 17
# speedup vs baseline: 1.4062x; 1.4062x over previous
"""Multi-head attention (B=2, S=2048, D=1024, H=16) on 8 TRN2 NeuronCores.

Sharding: DP=2 over batch x TP=4 over heads (4 heads/core).
Per core: QKV projections for its 256 output dims, attention for its 4
heads on its batch, row-parallel output projection producing a partial
[2048, 1024] bf16; host sums the 4 partials per batch and adds bo.

v2.1 layout strategy (per core):
  - host pre-transposes x (q/k/v) to xT [1024, 2048] bf16 and weights bf16
  - emit order tuned for overlap: k-proj, q-proj(first half), scores+exp
    for the first (ic,hp) chunk (ACT exp starts early), q-proj(rest),
    v-proj, then the attention steady loop with the output projection of
    each ic interleaved as soon as both head-pairs finish
  - qh/kh stored f32r (bf16 q/k caused max-err spikes), v/et/stacked/wo bf16
  - exp split: most j-tiles on ACT (exp, scale=1/8 folded); DVE_JS j-tiles
    per (hp,ic) on DVE via a two-term Schraudolph: two tensor_scalar ops
    fp32->int16 (RNE) whose bit patterns are bf16 encodings of
    ~2^t + 2^(t-0.5), summed with one GpSimd bf16 add (rel err ~0.5% rms,
    consistent denominators). DVE j-tiles are emitted early in each pair
    and consumed LAST in the attnV accumulation so their longer latency
    chain stays off the PE critical path.
  - all DMAs on the sync queue (GpSimd SWDGE is slow and must not be
    coupled with the et adds); GpSimd runs only the Schraudolph adds
  - attnV via vh_aug [128, 65] (ones column -> denominator row 64)
  - normalization: denominator rows copied to bf16, broadcast to 128 rows
    with two K=1 col-tiled matmuls, reciprocal_approx_fast on the
    broadcast, two muls into the bf16 `stacked` layout
"""
import numpy as np

B, S, D = 2, 2048, 1024
HEADS, DK = 16, 64
NCORES, DP, TP = 8, 2, 4
OPC = D // TP          # 256 output dims per core
HPC = HEADS // TP      # 4 heads per core
NDC = D // 128         # 8 contraction chunks
NST = S // 128         # 16 s-tiles (j tiles)
NIC = S // 512         # 4 i-chunks

# two-term Schraudolph constants (see sim_err.py calibration)
SCH_A = float(0.125 * 1.4426950408889634 * 128)
SCH_B1 = 16149.67
SCH_B2 = 16086.17
DVE_JS = (5, 10, 15)       # j-tiles whose exp runs on DVE (consumed last)

_cache = {}


def _build():
    import concourse.mybir as mybir
    import concourse.tile as tile
    from concourse import bacc

    F32 = mybir.dt.float32
    F32R = mybir.dt.float32r
    BF16 = mybir.dt.bfloat16
    I16 = mybir.dt.int16
    Exp = mybir.ActivationFunctionType.Exp
    Mult = mybir.AluOpType.mult
    Add = mybir.AluOpType.add

    nc = bacc.Bacc("TRN2", target_bir_lowering=False, debug=False)

    xq_d = nc.dram_tensor("xqt", [D, S], BF16, kind="ExternalInput")
    xk_d = nc.dram_tensor("xkt", [D, S], BF16, kind="ExternalInput")
    xv_d = nc.dram_tensor("xvt", [D, S], BF16, kind="ExternalInput")
    wq_d = nc.dram_tensor("wqt", [D, OPC], BF16, kind="ExternalInput")
    wk_d = nc.dram_tensor("wkt", [D, OPC], BF16, kind="ExternalInput")
    wv_d = nc.dram_tensor("wvt", [D, OPC], BF16, kind="ExternalInput")
    bq_d = nc.dram_tensor("bq", [2, 128, 1], F32, kind="ExternalInput")
    bk_d = nc.dram_tensor("bk", [2, 128, 1], F32, kind="ExternalInput")
    bv_d = nc.dram_tensor("bv", [128, OPC], F32, kind="ExternalInput")
    wo_d = nc.dram_tensor("wot", [2, 128, D], BF16, kind="ExternalInput")
    out_d = nc.dram_tensor("out", [S, D], BF16, kind="ExternalOutput")

    # attnV accumulation order: ACT-exp'd tiles first, DVE-exp'd last
    ACC_ORDER = [j for j in range(NST) if j not in DVE_JS] + list(DVE_JS)
    FIRST_J, LAST_J = ACC_ORDER[0], ACC_ORDER[-1]

    with tile.TileContext(nc) as tc:
        from contextlib import ExitStack
        es = ExitStack()
        with es:
            wp = es.enter_context(tc.tile_pool(name="wp", bufs=1))
            acts = es.enter_context(tc.tile_pool(name="acts", bufs=1))
            xp = es.enter_context(tc.tile_pool(name="xin", bufs=1))
            ep = es.enter_context(tc.tile_pool(name="ep", bufs=1))
            sps = es.enter_context(tc.tile_pool(name="sps", bufs=2, space="PSUM"))

            # constants
            ones164 = wp.tile([1, 64], BF16, name="ones164")
            nc.vector.memset(ones164[:], 1.0)

            # persistent activations
            qh_st = [acts.tile([128, S], F32R, name=f"qh{h}") for h in range(2)]
            kh_st = [acts.tile([128, S], F32R, name=f"kh{h}") for h in range(2)]
            # vh_aug: 16 j-tile blocks x 4 heads of [128, 65]; ones col at 64
            vh_all = acts.tile([128, NST * HPC * 65], BF16, name="vh_all")
            ones_cols = vh_all[:].rearrange("p (g c) -> p g c", c=65)[:, :, 64:65]
            nc.vector.memset(ones_cols, 1.0)
            stacked = [acts.tile([128, S], BF16, name=f"st{h}") for h in range(2)]

            def vh_ap(h, j):
                base = (j * HPC + h) * 65
                return vh_all[:, base:base + 65]

            def et_tile():
                return ep.tile([128, 1024], BF16, name="et", tag="et", bufs=20)

            def s12_tile():
                return ep.tile([128, 1024], I16, name="s12", tag="s12", bufs=6)

            def dma(dst, src):
                nc.sync.dma_start(dst, src)

            # ---------- phase helpers ----------
            def load_w(wd):
                wt = [wp.tile([128, OPC], BF16, name=f"w_{wd.name}_{i}")
                      for i in range(NDC)]
                for i in range(NDC):
                    dma(wt[i][:], wd.ap()[i * 128:(i + 1) * 128, :])
                return wt

            def load_x_chunks(xd, sc2):
                xt = [xp.tile([128, 1024], BF16, name="xt", tag="xt", bufs=16)
                      for _ in range(NDC)]
                for dc in range(NDC):
                    dma(xt[dc][:],
                        xd.ap()[dc * 128:(dc + 1) * 128,
                                sc2 * 1024:(sc2 + 1) * 1024])
                return xt

            def load_x_full(xd):
                xt = [xp.tile([128, S], BF16, name="xq", tag="xq", bufs=8)
                      for _ in range(NDC)]
                for dc in range(NDC):
                    dma(xt[dc][:], xd.ap()[dc * 128:(dc + 1) * 128, :])
                return xt

            def qk_proj(pool, xt, xoff, wt, bias, dest, sc2):
                # dest[hp][:, sc*512:(sc+1)*512] = wt.T @ xt_cols + bias
                for half in range(2):
                    sc = sc2 * 2 + half
                    lo = xoff + half * 512
                    for hp in range(2):
                        p = pool.tile([128, 512], F32, name="pp", tag="pp")
                        for dc in range(NDC):
                            nc.tensor.matmul(
                                p[:], wt[dc][:, hp * 128:(hp + 1) * 128],
                                xt[dc][:, lo:lo + 512],
                                start=(dc == 0), stop=(dc == NDC - 1))
                        nc.vector.tensor_scalar_add(
                            dest[hp][:, sc * 512:(sc + 1) * 512], p[:],
                            bias[hp][:])

            def scores_pair(hp, ic, j):
                sp = sps.tile([128, 1024], F32, name="sp", tag="sp")
                nc.tensor.matmul(
                    sp[:, 0:512], kh_st[hp][0:64, j * 128:(j + 1) * 128],
                    qh_st[hp][0:64, ic * 512:(ic + 1) * 512],
                    start=True, stop=True, tile_position=(0, 0))
                nc.tensor.matmul(
                    sp[:, 512:1024], kh_st[hp][64:128, j * 128:(j + 1) * 128],
                    qh_st[hp][64:128, ic * 512:(ic + 1) * 512],
                    start=True, stop=True, tile_position=(64, 0))
                return sp

            def exp_tile(sp, j):
                et = et_tile()
                if j in DVE_JS:
                    s1 = s12_tile()
                    s2 = s12_tile()
                    nc.vector.tensor_scalar(s1[:], sp[:], SCH_A, SCH_B1,
                                            Mult, Add)
                    nc.vector.tensor_scalar(s2[:], sp[:], SCH_A, SCH_B2,
                                            Mult, Add)
                    nc.gpsimd.tensor_tensor(
                        et[:], s1[:].bitcast(BF16), s2[:].bitcast(BF16), Add)
                else:
                    nc.scalar.activation(et[:], sp[:], Exp, scale=0.125)
                return et

            # emission order for a pair: DVE j-tiles early (their latency
            # chain hides behind ACT tiles): d0 a0 a1 d1 a2 a3 d2 a4 ... a12
            _dv = list(DVE_JS)
            _ac = [j for j in range(NST) if j not in DVE_JS]
            EMIT_ORDER = []
            _ai = 0
            for _di in range(len(_dv)):
                EMIT_ORDER.append(_dv[_di])
                EMIT_ORDER.extend(_ac[_ai:_ai + 2])
                _ai += 2
            EMIT_ORDER.extend(_ac[_ai:])

            def scores_exp_seq(hp, ic):
                return {j: exp_tile(scores_pair(hp, ic, j), j)
                        for j in EMIT_ORDER}

            def attn_v(av, hp, j, et):
                for h2 in range(2):
                    nc.tensor.matmul(
                        av[h2][0:DK + 1, :], vh_ap(hp * 2 + h2, j),
                        et[:, h2 * 512:(h2 + 1) * 512],
                        start=(j == FIRST_J), stop=(j == LAST_J),
                        skip_group_check=True)

            def norm(bp, av, hp, ic):
                # copy both heads' denominator rows (bf16), broadcast to 128
                # rows with two K=1 col-tiled matmuls, then reciprocal of the
                # broadcast (DVE time ~ free-dim only) and scale
                den = [ep.tile([1, 512], BF16, name="den", tag="den", bufs=4)
                       for _ in range(2)]
                for h2 in range(2):
                    nc.vector.tensor_copy(den[h2][:], av[h2][DK:DK + 1, :])
                r2 = bp.tile([128, 512], F32, name="r2", tag="mb")
                nc.tensor.matmul(r2[0:64, :], ones164[:], den[0][:],
                                 start=True, stop=True, tile_position=(0, 0))
                nc.tensor.matmul(r2[64:128, :], ones164[:], den[1][:],
                                 start=True, stop=True, tile_position=(0, 64))
                r2s = ep.tile([128, 512], F32, name="r2s", tag="r2s", bufs=2)
                nc.vector.reciprocal_approx_fast(r2s[:], r2[:])
                for h2 in range(2):
                    nc.vector.tensor_mul(
                        stacked[hp][h2 * 64:(h2 + 1) * 64,
                                    ic * 512:(ic + 1) * 512],
                        av[h2][0:DK, :], r2s[h2 * 64:(h2 + 1) * 64, :])

            def out_proj(bp, wo_t, ic):
                for it in range(ic * 4, ic * 4 + 4):
                    for mc in range(2):
                        po = bp.tile([128, 512], F32, name="po", tag="mb")
                        for hp in range(2):
                            nc.tensor.matmul(
                                po[:], stacked[hp][:, it * 128:(it + 1) * 128],
                                wo_t[hp][:, mc * 512:(mc + 1) * 512],
                                start=(hp == 0), stop=(hp == 1))
                        ot = ep.tile([128, 512], BF16, name="ot", tag="ot",
                                     bufs=4)
                        nc.vector.tensor_copy(ot[:], po[:])
                        dma(out_d.ap()[it * 128:(it + 1) * 128,
                                       mc * 512:(mc + 1) * 512], ot[:])

            # ================= phase A + prefill =================
            with tc.tile_pool(name="pps", bufs=3, space="PSUM") as pps:
                # k first (scores need all of kh): prioritize wk + xk sc2=0
                wk_t = load_w(wk_d)
                xk0 = load_x_chunks(xk_d, 0)
                bk_t = [wp.tile([128, 1], F32, name=f"bk{h}") for h in range(2)]
                for h in range(2):
                    dma(bk_t[h][:], bk_d.ap()[h])
                qk_proj(pps, xk0, 0, wk_t, bk_t, kh_st, 0)
                xk1 = load_x_chunks(xk_d, 1)
                qk_proj(pps, xk1, 0, wk_t, bk_t, kh_st, 1)

                wq_t = load_w(wq_d)
                xq = load_x_full(xq_d)
                bq_t = [wp.tile([128, 1], F32, name=f"bq{h}") for h in range(2)]
                for h in range(2):
                    dma(bq_t[h][:], bq_d.ap()[h])
                qk_proj(pps, xq, 0, wq_t, bq_t, qh_st, 0)

                # prefill: scores+exp for (ic=0, hp=0) while v-proj runs
                prefill_ets = scores_exp_seq(0, 0)

                # q second half (reuses the resident xq tiles)
                qk_proj(pps, xq, 1024, wq_t, bq_t, qh_st, 1)

                # v projection
                wv_t = load_w(wv_d)
                bv2 = wp.tile([128, OPC], F32, name="bv2")
                dma(bv2[:], bv_d.ap())
                for sc2 in range(2):
                    xt = load_x_chunks(xv_d, sc2)
                    for st8 in range(8):
                        st = sc2 * 8 + st8
                        pv = pps.tile([128, OPC], F32, name="pv", tag="pp")
                        for dc in range(NDC):
                            nc.tensor.matmul(
                                pv[:], xt[dc][:, st8 * 128:(st8 + 1) * 128],
                                wv_t[dc][:], start=(dc == 0),
                                stop=(dc == NDC - 1))
                        # scatter 4 heads' 64-dim slices into vh_all block st
                        dst = vh_all[:, st * HPC * 65:(st + 1) * HPC * 65]
                        dst = dst.rearrange("p (h c) -> p h c", h=HPC)[:, :, 0:64]
                        nc.vector.tensor_add(
                            dst, pv[:].rearrange("p (h c) -> p h c", h=HPC),
                            bv2[:].rearrange("p (h c) -> p h c", h=HPC))

            # wo (small, early in phase B)
            wo_t = [wp.tile([128, D], BF16, name=f"wo{h}") for h in range(2)]
            for h in range(2):
                dma(wo_t[h][:], wo_d.ap()[h])

            # ================= phase B/C steady loop =================
            # one 4-slot pool (1 bank/slot) shared by av accumulators, the
            # r2 broadcast, and the out-proj tiles: 4 banks + sps 4 = 8
            with tc.tile_pool(name="bp", bufs=4, space="PSUM") as bp:
                for ic in range(NIC):
                    for hp in range(2):
                        av = [bp.tile([DK + 1, 512], F32, name="av", tag="mb")
                              for _ in range(2)]
                        if ic == 0 and hp == 0:
                            for j in ACC_ORDER:
                                attn_v(av, hp, j, prefill_ets.pop(j))
                        else:
                            ets = {}
                            ei = [0]

                            def emit_up_to(n, hp=hp, ic=ic):
                                while ei[0] < min(n, NST):
                                    j = EMIT_ORDER[ei[0]]
                                    ets[j] = exp_tile(
                                        scores_pair(hp, ic, j), j)
                                    ei[0] += 1

                            for i, j in enumerate(ACC_ORDER):
                                emit_up_to(i + 4)
                                attn_v(av, hp, j, ets.pop(j))
                        norm(bp, av, hp, ic)
                    out_proj(bp, wo_t, ic)

    nc.compile()
    return nc


def _prep_inputs(q, k, v, Wq, bq, Wk, bk, Wv, bv, Wo, bo):
    import ml_dtypes
    f = np.float32
    bf = ml_dtypes.bfloat16
    xT = {}
    for g in range(DP):
        xT[("q", g)] = np.ascontiguousarray(np.asarray(q[g], f).T.astype(bf))
        xT[("k", g)] = np.ascontiguousarray(np.asarray(k[g], f).T.astype(bf))
        xT[("v", g)] = np.ascontiguousarray(np.asarray(v[g], f).T.astype(bf))
    Wq, Wk, Wv, Wo = (np.asarray(a, f) for a in (Wq, Wk, Wv, Wo))
    bq, bk, bv = (np.asarray(a, f) for a in (bq, bk, bv))
    in_maps = []
    for c in range(NCORES):
        g, r = divmod(c, TP)
        sl = slice(r * OPC, (r + 1) * OPC)
        in_maps.append({
            "xqt": xT[("q", g)], "xkt": xT[("k", g)], "xvt": xT[("v", g)],
            "wqt": np.ascontiguousarray(Wq[sl].T.astype(bf)),
            "wkt": np.ascontiguousarray(Wk[sl].T.astype(bf)),
            "wvt": np.ascontiguousarray(Wv[sl].T.astype(bf)),
            "bq": bq[sl].reshape(2, 128, 1),
            "bk": bk[sl].reshape(2, 128, 1),
            "bv": np.ascontiguousarray(np.broadcast_to(bv[sl], (128, OPC))),
            "wot": np.ascontiguousarray(Wo[:, sl].T).reshape(2, 128, D).astype(bf),
        })
    return in_maps


def kernel(q, k, v, Wq, bq, Wk, bk, Wv, bv, Wo, bo, _trace=False):
    from concourse.bass_utils import run_bass_kernel_spmd

    if "nc" not in _cache:
        _cache["nc"] = _build()
    nc = _cache["nc"]
    in_maps = _prep_inputs(q, k, v, Wq, bq, Wk, bk, Wv, bv, Wo, bo)
    res = run_bass_kernel_spmd(nc, in_maps, list(range(NCORES)), trace=_trace)
    _cache["last_exec_time_ns"] = res.exec_time_ns
    _cache["last_res"] = res
    parts = [res.results[c]["out"] for c in range(NCORES)]
    bo = np.asarray(bo, np.float32)
    out = np.empty((B, S, D), np.float32)
    for g in range(DP):
        acc = parts[g * TP].astype(np.float32)
        for r in range(1, TP):
            acc = acc + parts[g * TP + r].astype(np.float32)
        out[g] = acc + bo
    return out
